# revision 12
# baseline (speedup 1.0000x reference)
"""Trainium2 Bass kernel for nn_Attention (B=16,N=4096,C=1024,H=16,HD=64,Q=64).

Data-parallel over B across 8 NeuronCores (2 batches/core). Per batch the
attention is reassociated so no k/v tensors are materialized and no on-chip
transposes are needed:

  q^T = Wq @ x_q^T                      [(h,d)=1024, 64]
  G_h^T = Wk_h^T @ q_h                  G^T: [c=1024, (h,q)=1024]
  S^T   = x @ G^T   (per t-tile)        [t, (h,q)]   (contract c)
  p^T   = exp(S^T / 8)                  (softmax w/o max-sub: scores ~ +-5)
  u^T   = x^T(nat) @ p^T  (accum t)     [c, (h,q)]   (contract t)
  den   = ones @ pacc     (pacc: GpSimd p-sum over t)
  o_h^T = (Wv_h^T)^T @ u_h^T, scaled by 1/den at PSUM eviction
  y     = o^T.T @ Wproj^T + b           [64, 1024]   (contract (h,d))

Host feeds per core: x natural + x transposed, Wq^T, Wk, Wv^T, Wproj^T, b —
x/weights in bf16; all matmuls bf16 with fp32 PSUM accumulation, N=512.

Schedule notes:
- Each dma_start costs ~650ns of sync-engine issue time, so transfers are
  batched into ONE instruction per weight matrix / x block via 3D access
  patterns (rearrange).  q reads the first 64 token columns of xtt blk0,
  so no separate x_q feed is needed.
- DMA priority order: wq -> xtt(b0,blk0) -> wk -> xnt(b0,blk0) ->
  xtt(b1,blk0 dedicated) -> blk1 -> wv/wp -> stream.
- u accumulation runs in 1-block rounds (frees x slots a phase early so
  the next batch's stream prefetches across the batch boundary), and
  batch 1's first S block is emitted before batch 0's epilogue so the PE
  stays fed while the epilogue waits on vector evictions.
"""
import os
import numpy as np

SKIP = set(os.environ.get("ATT_SKIP", "").split(","))

B, N, C = 16, 4096, 1024
H, HD, QL = 16, 64, 64
BL = B // 8           # batches per core
CK = C // 128         # 8 c-tiles
TB = 512              # tokens per t-block
NBLK = N // TB        # 8 blocks
TPB = TB // 128       # 4 t-tiles per block
HQ = H * QL           # 1024
SCALE = HD ** -0.5

_CACHE = {}


def _build():
    import concourse.bass as bass
    import concourse.tile as tile
    from concourse import bacc, mybir

    f32 = mybir.dt.float32
    bf16 = mybir.dt.bfloat16
    EXP = mybir.ActivationFunctionType.Exp

    nc = bacc.Bacc("TRN2", target_bir_lowering=False, debug=False, num_devices=8)
    xn = nc.dram_tensor("xn", [BL, N, C], bf16, kind="ExternalInput").ap()
    xt = nc.dram_tensor("xt", [BL, C, N], bf16, kind="ExternalInput").ap()
    wq = nc.dram_tensor("wq", [C, C], bf16, kind="ExternalInput").ap()   # Wq^T
    wk = nc.dram_tensor("wk", [C, C], bf16, kind="ExternalInput").ap()   # Wk natural
    wv = nc.dram_tensor("wv", [C, C], bf16, kind="ExternalInput").ap()   # Wv^T
    wp = nc.dram_tensor("wp", [C, C], bf16, kind="ExternalInput").ap()   # Wproj^T
    bp = nc.dram_tensor("bp", [1, C], f32, kind="ExternalInput").ap()
    xq = nc.dram_tensor("xq", [BL, 128, CK * QL], bf16, kind="ExternalInput").ap()
    y = nc.dram_tensor("y", [BL, QL, C], f32, kind="ExternalOutput").ap()

    def wload(dst, src):
        # whole [C, C] weight -> [128, 8*1024] SBUF tile in ONE dma
        nc.sync.dma_start(
            dst[:].rearrange("p (k c) -> p k c", k=CK),
            src[:, :].rearrange("(k p) c -> p k c", p=128))

    with tile.TileContext(nc) as tc:
        with (
            tc.tile_pool(name="wpool", bufs=2) as wpool,
            tc.tile_pool(name="xpool", bufs=2) as xpool,
            tc.tile_pool(name="gpool", bufs=1) as gpool,
            tc.tile_pool(name="upool", bufs=1) as upool,
            tc.tile_pool(name="small", bufs=1) as small,
            tc.tile_pool(name="ptp", bufs=2) as ptp,
            tc.tile_pool(name="psa", bufs=4, space="PSUM") as psa,
            tc.tile_pool(name="psu", bufs=4, space="PSUM") as psu,
        ):
            ones32 = small.tile([128, 8], f32, tag="ones32")
            nc.gpsimd.memset(ones32[:], 1.0)
            bps = small.tile([128, C], bf16, tag="bps")
            nc.gpsimd.dma_start(bps[0:1, :], bp[:, :])
            bpf = small.tile([128, C], bf16, tag="bpf")
            nc.gpsimd.partition_broadcast(bpf[:], bps[0:1, :])

            def emit_block_dma(b, blk):
                xnt = xpool.tile([128, TPB * 1024], bf16, tag="xn")
                nc.sync.dma_start(
                    xnt[:].rearrange("p (i c) -> p i c", i=TPB),
                    xn[b, blk * TB:(blk + 1) * TB, :]
                    .rearrange("(i p) c -> p i c", p=128))
                xtt = xpool.tile([128, CK * TB], bf16, tag="xt")
                nc.sync.dma_start(
                    xtt[:].rearrange("p (k t) -> p k t", k=CK),
                    xt[b, :, blk * TB:(blk + 1) * TB]
                    .rearrange("(k p) t -> p k t", p=128))
                return xnt, xtt

            # ---------- DMA priority prefix ----------
            wt = wpool.tile([128, 8 * 1024], bf16, tag="w", name="wt_q")
            wload(wt, wq)
            xqts = []
            for b in range(BL):
                xqt = small.tile([128, CK * QL], bf16, tag="xqt", bufs=2,
                                 name=f"xqt{b}")
                nc.sync.dma_start(xqt[:], xq[b, :, :])
                xqts.append(xqt)
            wt2 = wpool.tile([128, 8 * 1024], bf16, tag="w", name="wt_k")
            wload(wt2, wk)
            xtt0 = xpool.tile([128, CK * TB], bf16, tag="xt")
            nc.sync.dma_start(
                xtt0[:].rearrange("p (k t) -> p k t", k=CK),
                xt[0, :, 0:TB].rearrange("(k p) t -> p k t", p=128))
            xnt0 = xpool.tile([128, TPB * 1024], bf16, tag="xn")
            nc.sync.dma_start(
                xnt0[:].rearrange("p (i c) -> p i c", i=TPB),
                xn[0, 0:TB, :].rearrange("(i p) c -> p i c", p=128))
            # dedicated early prefetch of b1 blk0's xtt (lives until b1 starts)
            xtb1 = xpool.tile([128, CK * TB], bf16, tag="xtb1", bufs=1,
                              name="xtb1")
            nc.sync.dma_start(
                xtb1[:].rearrange("p (k t) -> p k t", k=CK),
                xt[1, :, 0:TB].rearrange("(k p) t -> p k t", p=128))

            # ---------- both batches' q/G prologues ----------
            gts = []
            for b in range(BL):
                xq_src = xqts[b]
                # q^T chunks land directly into the block-diagonal layout:
                # chunk jc rows = heads (2jc, 2jc+1); all G matmuls K=128 base 0
                qbd = small.tile([128, 8 * 128], bf16, tag="qbd", bufs=2,
                                 name=f"qbd{b}")
                nc.gpsimd.memset(qbd[:], 0.0)
                for jc in range(8):
                    ps = psa.tile([128, 512], f32, tag="psa")
                    for ck in range(CK):
                        nc.tensor.matmul(
                            ps[:, 0:QL],
                            wt[:, ck * 1024 + jc * 128: ck * 1024 + (jc + 1) * 128],
                            xq_src[:, ck * QL:(ck + 1) * QL],
                            start=(ck == 0), stop=(ck == CK - 1))
                    nc.vector.tensor_copy(
                        qbd[0:64, jc * 128: jc * 128 + 64], ps[0:64, 0:QL])
                    nc.vector.tensor_copy(
                        qbd[64:128, jc * 128 + 64: (jc + 1) * 128], ps[64:128, 0:QL])

                # G^T [c,(h,q)] bf16: [128, CK*1024], c-tile ck at cols ck*1024
                gt = gpool.tile([128, CK * 1024], bf16, tag="gt", bufs=2,
                                name=f"gt{b}")
                for ck in range(CK):
                    for half in range(2):
                        ps = psa.tile([128, 512], f32, tag="psa")
                        for pp in range(4):
                            pair = half * 4 + pp
                            nc.tensor.matmul(
                                ps[:, pp * 128:(pp + 1) * 128],
                                wt2[:, pair * 1024 + ck * 128:
                                    pair * 1024 + (ck + 1) * 128],
                                qbd[:, pair * 128:(pair + 1) * 128],
                                start=True, stop=True)
                        nc.vector.tensor_copy(
                            gt[:, ck * 1024 + half * 512: ck * 1024 + (half + 1) * 512],
                            ps[:])
                gts.append(gt)

            # ---------- per-batch state ----------
            uts = [upool.tile([128, CK * 1024], f32, tag="ut", name=f"ut{b}")
                   for b in range(BL)]
            uns = [gpool.tile([128, CK * 1024], bf16, tag="un", name=f"un{b}")
                   for b in range(BL)]
            paccs = []
            for b in range(BL):
                pacc = small.tile([128, HQ], f32, tag="pacc", bufs=2,
                                  name=f"pacc{b}")
                nc.gpsimd.memset(pacc[:], 0.0)
                paccs.append(pacc)

            def emit_block_s(b, blk, xtt):
                gt, pacc = gts[b], paccs[b]
                ptc = ptp.tile([128, TPB * 1024], bf16, tag="ptc")
                for i in range(TPB):
                    for qh in range(2):
                        st = psa.tile([128, 512], f32, tag="psa")
                        for ck in range(CK):
                            nc.tensor.matmul(
                                st[:],
                                xtt[:, ck * TB + i * 128: ck * TB + (i + 1) * 128],
                                gt[:, ck * 1024 + qh * 512: ck * 1024 + (qh + 1) * 512],
                                start=(ck == 0), stop=(ck == CK - 1))
                        pslice = ptc[:, i * 1024 + qh * 512: i * 1024 + (qh + 1) * 512]
                        nc.scalar.activation(pslice, st[:], EXP, scale=SCALE)
                        pa = pacc[:, qh * 512:(qh + 1) * 512]
                        nc.gpsimd.tensor_add(pa, pslice, pa)
                return ptc

            def emit_block_u(b, blk, xnt, ptc):
                # 1-block u round: frees xnt a phase earlier than 2-block
                # rounds so the next batch's stream prefetches in time.
                ut, un = uts[b], uns[b]
                for qh in range(2):
                    for cq in range(2):
                        ups = [psu.tile([128, 512], f32, tag="ups",
                                        name=f"ups{b}_{blk}_{qh}_{cq}_{j}")
                               for j in range(4)]
                        for i in range(TPB):
                            for k4 in range(4):
                                ck = cq * 4 + k4
                                nc.tensor.matmul(
                                    ups[k4][:],
                                    xnt[:, i * 1024 + ck * 128: i * 1024 + (ck + 1) * 128],
                                    ptc[:, i * 1024 + qh * 512: i * 1024 + (qh + 1) * 512],
                                    start=(i == 0), stop=(i == TPB - 1))
                        for k4 in range(4):
                            ck = cq * 4 + k4
                            dst = ut[:, ck * 1024 + qh * 512: ck * 1024 + (qh + 1) * 512]
                            if blk == 0:
                                nc.vector.tensor_copy(dst, ups[k4][:])
                            elif blk == NBLK - 1:
                                nc.vector.tensor_add(
                                    un[:, ck * 1024 + qh * 512: ck * 1024 + (qh + 1) * 512],
                                    ups[k4][:], dst)
                            else:
                                nc.vector.tensor_add(dst, ups[k4][:], dst)

            def emit_den(b):
                # den/reciprocal/rdo only need pacc (complete at the last S
                # block's exp): emitted right after S(b, NBLK-1) so the slow
                # one-partition RECIPROCALs (~3.3us each) hide under the
                # final u round instead of serializing the epilogue.
                pacc = paccs[b]
                rd = small.tile([128, HQ], f32, tag="rd", name=f"rd{b}")
                for qh in range(2):
                    dnp = psa.tile([128, 512], f32, tag="psa", name=f"dnp{b}_{qh}")
                    nc.tensor.matmul(dnp[0:8, :], ones32[:],
                                     pacc[:, qh * 512:(qh + 1) * 512],
                                     start=True, stop=True)
                    nc.vector.reciprocal(rd[0:1, qh * 512:(qh + 1) * 512],
                                         dnp[0:1, :])
                rdf = small.tile([128, HQ], f32, tag="rdf", name=f"rdf{b}")
                nc.gpsimd.partition_broadcast(rdf[:], rd[0:1, :])
                # per-head-pair reciprocal layout for the oT scale:
                # rdo[p, jc*64+qq] = 1/d[(2jc + p//64)*64 + qq]
                rdo = small.tile([128, 8 * QL], f32, tag="rdo", name=f"rdo{b}")
                for jc in range(8):
                    nc.vector.tensor_copy(
                        rdo[0:64, jc * QL:(jc + 1) * QL],
                        rdf[0:64, (2 * jc) * QL:(2 * jc + 1) * QL])
                    nc.vector.tensor_copy(
                        rdo[64:128, jc * QL:(jc + 1) * QL],
                        rdf[64:128, (2 * jc + 1) * QL:(2 * jc + 2) * QL])
                return rdo

            def emit_epilogue(b, rdo):
                un = uns[b]
                oT = small.tile([128, 8 * QL], bf16, tag="oT", name=f"oT{b}")
                for jc in range(8):  # head pair (2jc, 2jc+1)
                    ps = psa.tile([128, 512], f32, tag="psa")
                    for sub in range(2):
                        h = jc * 2 + sub
                        ucol = (h // 8) * 512 + (h % 8) * 64
                        for ck in range(CK):
                            nc.tensor.matmul(
                                ps[sub * 64:(sub + 1) * 64, 0:QL],
                                wt3[:, ck * 1024 + h * 64: ck * 1024 + (h + 1) * 64],
                                un[:, ck * 1024 + ucol: ck * 1024 + ucol + 64],
                                start=(ck == 0), stop=(ck == CK - 1),
                                tile_position=(0, sub * 64))
                    nc.vector.tensor_mul(oT[:, jc * QL:(jc + 1) * QL],
                                         ps[:, 0:QL], rdo[:, jc * QL:(jc + 1) * QL])

                ys = small.tile([128, C], f32, tag="ys", name=f"ys{b}")
                for half in range(2):
                    ps = psa.tile([128, 512], f32, tag="psa")
                    for jc in range(8):
                        nc.tensor.matmul(
                            ps[0:QL, :],
                            oT[:, jc * QL:(jc + 1) * QL],
                            wt4[:, jc * 1024 + half * 512: jc * 1024 + (half + 1) * 512],
                            start=(jc == 0), stop=(jc == 7))
                    nc.vector.tensor_add(
                        ys[0:QL, half * 512:(half + 1) * 512], ps[0:QL, :],
                        bpf[0:QL, half * 512:(half + 1) * 512])
                nc.sync.dma_start(y[b, :, :], ys[0:QL, :])

            # ---------- t-loops ----------
            if "tloop" not in SKIP:
                # batch 0
                ptc = emit_block_s(0, 0, xtt0)
                nxt = emit_block_dma(0, 1)

                # epilogue weights: DMA priority after blk0/1 so the first S
                # matmuls aren't starved behind them
                wt3 = wpool.tile([128, 8 * 1024], bf16, tag="w", name="wt_v")
                wload(wt3, wv)
                wt4 = wpool.tile([128, 8 * 1024], bf16, tag="w", name="wt_p")
                wload(wt4, wp)

                emit_block_u(0, 0, xnt0, ptc)
                xn_cur, xt_cur = nxt
                for blk in range(1, NBLK):
                    ptc = emit_block_s(0, blk, xt_cur)
                    if blk == NBLK - 1:
                        rdo0 = emit_den(0)
                    nxt = emit_block_dma(0, blk + 1) if blk + 1 < NBLK else None
                    emit_block_u(0, blk, xn_cur, ptc)
                    if nxt is not None:
                        xn_cur, xt_cur = nxt

                # batch 1 block 0 S-phase before batch 0's epilogue: keeps the
                # PE fed while the epilogue waits on vector evictions.
                xnt_b1 = xpool.tile([128, TPB * 1024], bf16, tag="xn")
                nc.sync.dma_start(
                    xnt_b1[:].rearrange("p (i c) -> p i c", i=TPB),
                    xn[1, 0:TB, :].rearrange("(i p) c -> p i c", p=128))
                ptc = emit_block_s(1, 0, xtb1)
                nxt = emit_block_dma(1, 1)
                emit_epilogue(0, rdo0)
                emit_block_u(1, 0, xnt_b1, ptc)
                xn_cur, xt_cur = nxt
                for blk in range(1, NBLK):
                    ptc = emit_block_s(1, blk, xt_cur)
                    if blk == NBLK - 1:
                        rdo1 = emit_den(1)
                    nxt = emit_block_dma(1, blk + 1) if blk + 1 < NBLK else None
                    emit_block_u(1, blk, xn_cur, ptc)
                    if nxt is not None:
                        xn_cur, xt_cur = nxt
                emit_epilogue(1, rdo1)
            else:
                wt3 = wpool.tile([128, 8 * 1024], bf16, tag="w", name="wt_v")
                wload(wt3, wv)
                wt4 = wpool.tile([128, 8 * 1024], bf16, tag="w", name="wt_p")
                wload(wt4, wp)
                emit_epilogue(0, emit_den(0))
                emit_epilogue(1, emit_den(1))

    nc.compile()
    return nc


def get_nc():
    if "nc" not in _CACHE:
        _CACHE["nc"] = _build()
    return _CACHE["nc"]


def make_in_maps(x, Wq, Wk, Wv, Wproj, bproj):
    import ml_dtypes
    bf = ml_dtypes.bfloat16
    x = np.ascontiguousarray(x, dtype=np.float32)
    xt32 = np.ascontiguousarray(x.transpose(0, 2, 1))
    xtb = xt32.astype(bf)
    xnb = x.astype(bf)
    # xq swizzled [B, 128, CK*QL]: xq[b, p, k*QL+q] = x^T[b, k*128+p, q]
    xqb = (xt32[:, :, 0:QL].reshape(B, CK, 128, QL).transpose(0, 2, 1, 3)
           .reshape(B, 128, CK * QL).astype(bf))
    wqb = np.ascontiguousarray(np.asarray(Wq, dtype=np.float32).T).astype(bf)
    wkb = np.ascontiguousarray(np.asarray(Wk, dtype=np.float32)).astype(bf)
    wvb = np.ascontiguousarray(np.asarray(Wv, dtype=np.float32).T).astype(bf)
    wpb = np.ascontiguousarray(np.asarray(Wproj, dtype=np.float32).T).astype(bf)
    bpf = np.ascontiguousarray(np.asarray(bproj, dtype=np.float32).reshape(1, C))
    in_maps = []
    for core in range(8):
        s = slice(core * BL, (core + 1) * BL)
        in_maps.append({
            "xn": np.ascontiguousarray(xnb[s]),
            "xt": np.ascontiguousarray(xtb[s]),
            "xq": np.ascontiguousarray(xqb[s]),
            "wq": wqb, "wk": wkb, "wv": wvb, "wp": wpb, "bp": bpf,
        })
    return in_maps


def kernel(x, Wq, Wk, Wv, Wproj, bproj):
    from concourse import bass_utils
    nc = get_nc()
    in_maps = make_in_maps(x, Wq, Wk, Wv, Wproj, bproj)
    res = bass_utils.run_bass_kernel_spmd(nc, in_maps, core_ids=list(range(8)))
    out = np.concatenate([res.results[i]["y"] for i in range(8)], axis=0)
    return out.astype(np.float32)
